# revision 2
# baseline (speedup 1.0000x reference)
"""Trainium2 Bass kernel for nn_BigAttention (weight-norm MLP + softmax-over-k).

Math (per the reference):
    W1e = g1 * W1 / ||W1||_F          [1024, 3072]
    W2e = g2 * W2 / ||W2||_F          [1, 1024]
    hv  = v @ W1e[:, :2048].T         [B,K,N,1024]
    hq  = q @ W1e[:, 2048:].T         [B,K,1024]
    joint  = relu(hv + hq + b1)
    logits = joint @ W2e.T  (+ b2, which cancels in the softmax over k)
    out = softmax(logits, axis=K)     [B,K,N,1]

Sharding: data-parallel over batch, 8 batches per core; weights replicated.

Precision: the big contractions run in fp8e4m3 with the PE's DoubleRow mode
(2 fp8 contraction chunks per matmul -> ~2x the bf16 matmul rate).  W1e's
entries are ~5.6e-4 rms (weight_norm divides by ||W1||_F ~ 35), far below
fp8's normal range, so the host scales W1e by S=1024 before quantizing and
folds 1/S into the fp32 W2 epilogue (relu commutes with the positive scale).
Measured end-to-end max rel err ~3e-3 (tolerance 2e-2).

Per-core device program (rows r = (b_local, k, n) flattened, R = 8*12*36 = 3456):
  - hq[96, 1024] via 4 DoubleRow matmuls per 512-half (q^T/W1q^T fp8 pairs),
    b1*S folded in on the DVE eviction; result stored bf16.
  - main: per 128-row tile, PSUM[row, hidden 1024] accumulates 8 DoubleRow
    fp8 matmuls (v^T pair stationary, W1v^T pair moving) plus ONE bf16
    one-hot matmul per 512-half that adds hq[bk(row), :].
  - epilogue per tile: one DVE scalar_tensor_tensor computes
    (PSUM max 0) * w2_broadcast with accum_out = per-row sum = the logit.
  - softmax over k: logits go [128, 27] -> StreamTranspose -> linear DRAM ->
    [96 (b,k), 36 n] SBUF; exp on ACT; the per-(b,n) sum and its broadcast
    back over k are two tiny one-hot matmuls on the PE; final scale on DVE;
    one strided DMA writes the [8,12,36,1] output slice.

All heavy inputs are host-repacked "partition-major" so every big DMA is 128
contiguous runs (one per partition) instead of thousands of thin descriptors.
Weight DMAs ride the scalar-engine HWDGE ring, v DMAs the sync ring, so
descriptor generation overlaps.
"""

import ml_dtypes
import numpy as np

import concourse.bacc as bacc
import concourse.mybir as mybir
import concourse.tile as tile
from concourse.bass_utils import run_bass_kernel_spmd

F32 = mybir.dt.float32
NCORES = 8
B, K, N = 64, 12, 36
VD, QD, HID = 2048, 1024, 1024
BL = B // NCORES              # local batches per core
R = BL * K * N                # 3456 rows per core
BK = BL * K                   # 96 (b,k) groups per core
CC = VD // 128                # 16 contraction chunks over v-dim
QC = QD // 128                # 8 contraction chunks over q-dim
RC = 384                      # rows per DMA chunk (9 chunks)
NCH = R // RC
RT = 128                      # rows per PSUM tile
NT = RC // RT
NRT = R // RT                 # 27 row tiles
VSPLIT = 16                   # v-chunk DMA granularity (cc chunks per DMA)
SCALE = 1024.0                # host-side W1 scale so fp8 sees O(1) weights

_NC_CACHE = None

F8 = mybir.dt.float8e4
BF16 = mybir.dt.bfloat16
DR = mybir.MatmulPerfMode.DoubleRow


def _build_nc():
    nc = bacc.Bacc("TRN2", target_bir_lowering=False, debug=False,
                   num_devices=NCORES)

    def mm(out, lhsT, rhs, **kw):
        nc.tensor.matmul(out, lhsT, rhs, **kw)

    w1vt = nc.dram_tensor("w1vt", [128, CC, HID], F8, kind="ExternalInput").ap()
    # qt and W1q^T packed along the free dim: [:, cq, 0:96]=q^T, [:, cq, 96:1120]=W1q^T
    qtwq = nc.dram_tensor("qtwq", [128, QC, BK + HID], F8, kind="ExternalInput").ap()
    # bf16 one-hot row-selection matrix for the hq-add closers
    oneh_d = nc.dram_tensor("oneh", [BK, R], BF16, kind="ExternalInput").ap()
    # bf16 constants: [:, 0:1024]=w2/S bcast, [0:96, 1024:2048]=b1*S replicated
    packb = nc.dram_tensor("packb", [128, HID + HID], BF16, kind="ExternalInput").ap()
    # fp32 softmax selectors: [0:96, 0:8]=selb, [0:8, 8:104]=selb^T
    packf = nc.dram_tensor("packf", [128, BL + BK], F32, kind="ExternalInput").ap()
    # v is split: the first two chunks ride with the weights at the front of
    # the upload order; the bulk uploads last, hidden under early compute.
    vth = nc.dram_tensor("vth", [2, 128, CC, RC], F8, kind="ExternalInput").ap()
    vtr = nc.dram_tensor("vtr", [NCH - 2, 128, CC, RC], F8, kind="ExternalInput").ap()
    out = nc.dram_tensor("out", [BL, K, N, 1], F32, kind="ExternalOutput").ap()

    MAX = mybir.AluOpType.max
    MULT = mybir.AluOpType.mult
    BYPASS = mybir.AluOpType.bypass
    ADD = mybir.AluOpType.add

    with tile.TileContext(nc) as tc:
        with tc.tile_pool(name="const", bufs=1) as cpool, \
             tc.tile_pool(name="wv", bufs=1) as wvpool, \
             tc.tile_pool(name="vtp", bufs=2) as vtpool, \
             tc.tile_pool(name="work", bufs=3) as work, \
             tc.tile_pool(name="small", bufs=1) as small, \
             tc.tile_pool(name="dram", bufs=1, space="DRAM") as dpool, \
             tc.tile_pool(name="psum", bufs=4, space="PSUM") as pspool:

            # ---- startup set as fat DMAs (the Tile runtime can only track
            # ~8 outstanding DMA completions; many small DMAs serialize and
            # starve the PE). Issue order matches consumption order.
            def vt_chunk_tiles(ch):
                src_ap = vth[ch] if ch < 2 else vtr[ch - 2]
                tiles = []
                for j in range(CC // VSPLIT):
                    t = vtpool.tile([128, VSPLIT, RC], F8, tag=f"vt{j}")
                    nc.sync.dma_start(
                        out=t, in_=src_ap[:, j * VSPLIT:(j + 1) * VSPLIT, :])
                    tiles.append(t)
                return tiles

            vt_cur = vt_chunk_tiles(0)

            packb_s = cpool.tile([128, HID + HID], BF16)
            nc.sync.dma_start(out=packb_s, in_=packb)

            qtwq_s = cpool.tile([128, QC, BK + HID], F8)
            nc.sync.dma_start(out=qtwq_s, in_=qtwq)

            vt_next = vt_chunk_tiles(1)

            oneh_s = cpool.tile([BK, R], BF16)
            nc.sync.dma_start(out=oneh_s, in_=oneh_d)

            packf_s = cpool.tile([128, BL + BK], F32)
            nc.sync.dma_start(out=packf_s, in_=packf)

            WG = 8  # wv group size (cc chunks per DMA)
            wv_g = []
            for j in range(CC // WG):
                t = wvpool.tile([128, WG, HID], F8, tag=f"wvg{j}")
                nc.scalar.dma_start(out=t, in_=w1vt[:, j * WG:(j + 1) * WG, :])
                wv_g.append(t)

            w2b_s = packb_s[:, 0:HID]
            b1b_s = packb_s[0:BK, HID:HID + HID]
            selb_s = packf_s[0:BK, 0:BL]
            selbt_s = packf_s[0:BL, BL:BL + BK]

            # per-row logits, laid out [p, rt] with row = rt*128 + p, split
            # into two tiles so the first half's DRAM flush hides under the
            # main loop. 32 columns (StreamTranspose needs 32x32 blocks).
            NRT_A = 18   # 18*128 rows = 64 (b,k) groups — a 32-aligned bk split
            ls_a = cpool.tile([128, 32], F32)
            nc.vector.memset(ls_a, 0.0)
            ls_b = cpool.tile([128, 32], F32)
            nc.vector.memset(ls_b, 0.0)
            lg = dpool.tile([R], F32)
            lg2 = lg.rearrange("(t p) -> t p", t=NRT, p=128)

            def flush_logits(ls, ls_t_name, t0, t1):
                # ls[p, t - t0] holds L[t*128 + p] for t in [t0, t1)
                ls_t = cpool.tile([128, 32], F32, name=ls_t_name)
                nc.vector.transpose(ls_t, ls)
                for i in range(4):
                    eng = nc.sync if i % 2 == 0 else nc.scalar
                    eng.dma_start(
                        out=lg2[t0:t1, 32 * i:32 * i + 32],
                        in_=ls_t[32 * i:32 * i + (t1 - t0), :])

            hq_s = cpool.tile([BK, HID], BF16)
            s96 = small.tile([BK, N], F32)
            e96 = small.tile([BK, N], F32)
            sums_ps = pspool.tile([BL, N], F32, tag="sm", bufs=2)

            PV = VSPLIT // 2   # DoubleRow cc-pairs per v DMA tile
            PW = WG // 2       # DoubleRow cc-pairs per wv group

            def emit_vmms(t, ps):
                for cp in range(CC // 2):
                    lhsT = vt_cur[cp // PV][:, (cp % PV) * 2:(cp % PV) * 2 + 2,
                                            t * RT:(t + 1) * RT]
                    wvc = wv_g[cp // PW][:, (cp % PW) * 2:(cp % PW) * 2 + 2, :]
                    mm(ps[:, 0:512], lhsT, wvc[:, :, 0:512],
                       start=(cp == 0), stop=False, perf_mode=DR)
                    mm(ps[:, 512:1024], lhsT, wvc[:, :, 512:1024],
                       start=(cp == 0), stop=False, perf_mode=DR)

            def emit_closer(rt, ps):
                oh = oneh_s[:, rt * RT:(rt + 1) * RT]
                mm(ps[:, 0:512], oh, hq_s[:, 0:512], start=False, stop=True)
                mm(ps[:, 512:1024], oh, hq_s[:, 512:1024], start=False, stop=True)
                relu_w2 = work.tile([128, HID], F32, tag="relu_w2")
                ls, col = (ls_a, rt) if rt < NRT_A else (ls_b, rt - NRT_A)
                nc.vector.scalar_tensor_tensor(
                    out=relu_w2, in0=ps, scalar=0.0, in1=w2b_s,
                    op0=MAX, op1=MULT,
                    accum_out=ls[:, col:col + 1])
                if rt == NRT_A - 1:
                    # flush + start the softmax head for bk rows 0:64 while
                    # the main loop still runs
                    flush_logits(ls_a, "ls_ta", 0, NRT_A)
                    nc.sync.dma_start(
                        out=s96[0:64, :],
                        in_=lg.rearrange("(bk n) -> bk n", n=N)[0:64, :])
                    nc.scalar.activation(e96[0:64, :], s96[0:64, :],
                                         mybir.ActivationFunctionType.Exp)
                    mm(sums_ps, selb_s[0:64, :], e96[0:64, :],
                       start=True, stop=False)

            # ---- chunk 0: v-matmuls for tiles 0..2 first, then hq (its DMAs
            # arrive under the v work), then the deferred closers.
            ps0 = []
            for t in range(NT):
                ps = pspool.tile([128, HID], F32, tag="ps", bufs=3)
                emit_vmms(t, ps)
                ps0.append(ps)

            # hq[bk, h] = q @ W1q^T (+ b1*S on eviction), via fp8 DoubleRow
            hq_ps = [pspool.tile([BK, 512], F32, tag="sm", bufs=2,
                                 name=f"hq_ps{i}") for i in range(2)]
            for half in range(2):
                hs = slice(half * 512, (half + 1) * 512)
                for cp in range(QC // 2):
                    mm(hq_ps[half],
                       qtwq_s[:, 2 * cp:2 * cp + 2, 0:BK],
                       qtwq_s[:, 2 * cp:2 * cp + 2,
                              BK + half * 512:BK + (half + 1) * 512],
                       start=(cp == 0), stop=(cp == QC // 2 - 1),
                       perf_mode=DR)
                # psum -> SBUF with the b1*S row added (b1b is b1*S replicated
                # across the 96 partitions host-side)
                nc.vector.scalar_tensor_tensor(
                    out=hq_s[:, hs], in0=hq_ps[half], scalar=0.0,
                    in1=b1b_s[:, hs],
                    op0=BYPASS, op1=ADD)

            for t in range(NT):
                emit_closer(t, ps0[t])
            vt_cur = vt_next

            # ---- chunks 1..8
            for ch in range(1, NCH):
                if ch + 1 < NCH:
                    vt_next = vt_chunk_tiles(ch + 1)
                for t in range(NT):
                    rt = ch * NT + t
                    ps = pspool.tile([128, HID], F32, tag="ps", bufs=3)
                    emit_vmms(t, ps)
                    emit_closer(rt, ps)
                vt_cur = vt_next

            # ---- flush remaining logits, finish the softmax
            flush_logits(ls_b, "ls_tb", NRT_A, NRT)
            nc.sync.dma_start(
                out=s96[64:BK, :],
                in_=lg.rearrange("(bk n) -> bk n", n=N)[64:BK, :])
            nc.scalar.activation(e96[64:BK, :], s96[64:BK, :],
                                 mybir.ActivationFunctionType.Exp)
            mm(sums_ps, selb_s[64:BK, :], e96[64:BK, :],
               start=False, stop=True)
            rcp = small.tile([BL, N], F32)
            nc.vector.reciprocal(rcp, sums_ps)
            rexp_ps = pspool.tile([BK, N], F32, tag="sm", bufs=2)
            mm(rexp_ps, selbt_s, rcp, start=True, stop=True)
            w96 = small.tile([BK, N], F32)
            nc.vector.scalar_tensor_tensor(
                out=w96, in0=e96, scalar=0.0, in1=rexp_ps,
                op0=BYPASS, op1=MULT)
            nc.sync.dma_start(
                out=out.rearrange("b k n o -> (b k) (n o)"), in_=w96)

    nc.compile()
    return nc


def _get_nc():
    global _NC_CACHE
    if _NC_CACHE is None:
        _NC_CACHE = _build_nc()
    return _NC_CACHE


def _prepare_in_maps(inputs):
    v = np.asarray(inputs["v"], dtype=np.float32)
    q = np.asarray(inputs["q"], dtype=np.float32)
    W1 = np.asarray(inputs["W1"], dtype=np.float32)
    g1 = np.float64(np.asarray(inputs["g1"]))
    b1 = np.asarray(inputs["b1"], dtype=np.float32)
    W2 = np.asarray(inputs["W2"], dtype=np.float32)
    g2 = np.float64(np.asarray(inputs["g2"]))
    # b2 is a scalar added to every logit -> cancels in softmax over k.

    W1e = ((g1 / np.linalg.norm(W1.astype(np.float64))) * W1).astype(np.float32)
    W2e = ((g2 / np.linalg.norm(W2.astype(np.float64))) * W2).astype(np.float32)

    BF = ml_dtypes.bfloat16
    FP8 = ml_dtypes.float8_e4m3
    W1s = (W1e * SCALE).astype(np.float32)
    # partition-major repacks: [..., 128 p, chunk, inner]
    w1vt = np.ascontiguousarray(                       # [128, 16, 1024]
        W1s[:, :VD].T.reshape(CC, 128, HID).transpose(1, 0, 2)).astype(FP8)
    w1qt = W1s[:, VD:].T.reshape(QC, 128, HID).transpose(1, 0, 2)  # [128, 8, 1024]
    r = np.arange(R)
    oneh = (np.arange(BK)[:, None] == (r // N)[None, :]).astype(BF)
    selb = (np.arange(BL)[None, :] == (np.arange(BK) // K)[:, None]).astype(np.float32)

    packb = np.zeros((128, HID + HID), dtype=BF)
    packb[:, 0:HID] = (W2e.reshape(1, HID) / SCALE).astype(BF)
    packb[0:BK, HID:HID + HID] = (b1.reshape(1, HID) * SCALE).astype(BF)
    packf = np.zeros((128, BL + BK), dtype=np.float32)
    packf[0:BK, 0:BL] = selb
    packf[0:BL, BL:BL + BK] = selb.T

    shared = dict(w1vt=w1vt, oneh=oneh, packb=packb, packf=packf)
    in_maps = []
    for c in range(NCORES):
        vl = v[c * BL:(c + 1) * BL].reshape(R, VD)
        # vt[ch, p, cc, r_in_chunk] = v[ch*RC + r, cc*128 + p]
        vt4 = np.ascontiguousarray(
            vl.T.reshape(CC, 128, NCH, RC).transpose(2, 1, 0, 3)).astype(FP8)
        ql = q[c * BL:(c + 1) * BL].reshape(BK, QD)
        qt3 = ql.T.reshape(QC, 128, BK).transpose(1, 0, 2)   # [128, 8, 96]
        qtwq = np.concatenate([qt3, w1qt], axis=2).astype(FP8)  # [128, 8, 1120]
        in_maps.append(dict(vth=np.ascontiguousarray(vt4[:2]),
                            vtr=np.ascontiguousarray(vt4[2:]),
                            qtwq=np.ascontiguousarray(qtwq), **shared))
    return in_maps


def kernel(**inputs) -> np.ndarray:
    in_maps = _prepare_in_maps(inputs)
    nc = _get_nc()
    res = run_bass_kernel_spmd(nc, in_maps, list(range(NCORES)))
    outs = [res.results[c]["out"].reshape(BL, K, N, 1) for c in range(NCORES)]
    return np.concatenate(outs, axis=0)


# revision 9
# speedup vs baseline: 1.0073x; 1.0073x over previous
"""Trainium2 Bass kernel for nn_BigAttention (weight-norm MLP + softmax-over-k).

Math (per the reference):
    W1e = g1 * W1 / ||W1||_F          [1024, 3072]
    W2e = g2 * W2 / ||W2||_F          [1, 1024]
    hv  = v @ W1e[:, :2048].T         [B,K,N,1024]
    hq  = q @ W1e[:, 2048:].T         [B,K,1024]
    joint  = relu(hv + hq + b1)
    logits = joint @ W2e.T  (+ b2, which cancels in the softmax over k)
    out = softmax(logits, axis=K)     [B,K,N,1]

Sharding: data-parallel over batch, 8 batches per core; weights replicated.

Precision: the big contractions run in fp8e4m3 with the PE's DoubleRow mode
(2 fp8 contraction chunks per matmul -> ~2x the bf16 matmul rate).  W1e's
entries are ~5.6e-4 rms (weight_norm divides by ||W1||_F ~ 35), far below
fp8's normal range, so the host scales W1e by S=1024 before quantizing and
folds 1/S into the fp32 W2 epilogue (relu commutes with the positive scale).
Measured end-to-end max rel err ~3e-3 (tolerance 2e-2).

Per-core device program (rows r = (b_local, k, n) flattened, R = 8*12*36 = 3456):
  - hq[96, 1024] via 4 DoubleRow matmuls per 512-half (q^T/W1q^T fp8 pairs),
    b1*S folded in on the DVE eviction; result stored bf16.  Emitted first so
    the PE starts as soon as the (first-uploaded) qtwq tile lands.
  - main: per 128-row tile, PSUM[row, hidden 1024] accumulates 8 DoubleRow
    fp8 matmuls (v^T pair stationary, W1v^T pair moving) plus ONE bf16
    one-hot matmul per 512-half that adds hq[bk(row), :].
  - epilogue per tile: one DVE scalar_tensor_tensor computes
    (PSUM max 0) * w2_broadcast with accum_out = per-row sum = the logit.
  - softmax over k: logits collect in [128 p, tile] sbuf tiles; 4 DVE 32x32
    block transposes build a true [tile, 128] transpose, then ONE SBUF->SBUF
    DMA re-groups rows into [96 (b,k), 36 n] (the element streams match:
    both walk r = tile*128+p = bk*36+n sequentially).  exp on ACT; per-(b,n)
    sum and its broadcast back over k are two tiny f32r one-hot matmuls on
    the PE; final scale on DVE; one DMA writes the [8,12,36,1] output slice.

Startup bytes are minimized: the one-hot matrix is generated on GpSimd
(memset + two affine_selects), and the w2/b1 broadcast rows are uploaded
once [1, 2048] and partition-broadcast on GpSimd.  Weight DMAs ride the
scalar-engine ring, v/q DMAs the sync ring, so descriptor generation
overlaps; upload order matches consumption order (qtwq -> v chunk 0 -> ...).
"""

import ml_dtypes
import numpy as np

import concourse.bacc as bacc
import concourse.mybir as mybir
import concourse.tile as tile
from concourse.bass_utils import run_bass_kernel_spmd

F32 = mybir.dt.float32
F32R = mybir.dt.float32r
NCORES = 8
B, K, N = 64, 12, 36
VD, QD, HID = 2048, 1024, 1024
BL = B // NCORES              # local batches per core
R = BL * K * N                # 3456 rows per core
BK = BL * K                   # 96 (b,k) groups per core
CC = VD // 128                # 16 contraction chunks over v-dim
QC = QD // 128                # 8 contraction chunks over q-dim
RC = 384                      # rows per DMA chunk (9 chunks)
NCH = R // RC
RT = 128                      # rows per PSUM tile
NT = RC // RT
NRT = R // RT                 # 27 row tiles
VSPLIT = 16                   # v-chunk DMA granularity (cc chunks per DMA)
SCALE = 1024.0                # host-side W1 scale so fp8 sees O(1) weights

_NC_CACHE = None

F8 = mybir.dt.float8e4
BF16 = mybir.dt.bfloat16
DR = mybir.MatmulPerfMode.DoubleRow


def _build_nc():
    nc = bacc.Bacc("TRN2", target_bir_lowering=False, debug=False,
                   num_devices=NCORES)

    def mm(out, lhsT, rhs, **kw):
        nc.tensor.matmul(out, lhsT, rhs, **kw)

    w1vt = nc.dram_tensor("w1vt", [128, CC, HID], F8, kind="ExternalInput").ap()
    # qt and W1q^T packed along the free dim: [:, cq, 0:96]=q^T, [:, cq, 96:1120]=W1q^T
    qtwq = nc.dram_tensor("qtwq", [128, QC, BK + HID], F8, kind="ExternalInput").ap()
    # single row: [0, 0:1024]=w2/S, [0, 1024:2048]=b1*S (partition-broadcast on dev)
    packc = nc.dram_tensor("packc", [1, HID + HID], BF16, kind="ExternalInput").ap()
    # f32r softmax selectors: [0:96, 0:8]=selb, [0:8, 8:104]=selb^T
    packf = nc.dram_tensor("packf", [128, BL + BK], F32R, kind="ExternalInput").ap()
    # v is split: the first two chunks ride with the weights at the front of
    # the upload order; the bulk uploads last, hidden under early compute.
    vth = nc.dram_tensor("vth", [2, 128, CC, RC], F8, kind="ExternalInput").ap()
    vtr = nc.dram_tensor("vtr", [NCH - 2, 128, CC, RC], F8, kind="ExternalInput").ap()
    out = nc.dram_tensor("out", [BL, K, N, 1], F32, kind="ExternalOutput").ap()

    MAX = mybir.AluOpType.max
    MULT = mybir.AluOpType.mult
    BYPASS = mybir.AluOpType.bypass
    ADD = mybir.AluOpType.add

    with tile.TileContext(nc) as tc:
        with tc.tile_pool(name="const", bufs=1) as cpool, \
             tc.tile_pool(name="wv", bufs=1) as wvpool, \
             tc.tile_pool(name="vtp", bufs=2) as vtpool, \
             tc.tile_pool(name="work", bufs=3) as work, \
             tc.tile_pool(name="small", bufs=1) as small, \
             tc.tile_pool(name="dram", bufs=1, space="DRAM") as dpool, \
             tc.tile_pool(name="psum", bufs=4, space="PSUM") as pspool:

            # ---- startup uploads, issue order == consumption order.
            qtwq_s = cpool.tile([128, QC, BK + HID], F8)
            nc.sync.dma_start(out=qtwq_s, in_=qtwq)

            WG = 8  # wv group size (cc chunks per DMA)
            wv_g = []
            for j in range(CC // WG):
                t = wvpool.tile([128, WG, HID], F8, tag=f"wvg{j}")
                nc.scalar.dma_start(out=t, in_=w1vt[:, j * WG:(j + 1) * WG, :])
                wv_g.append(t)

            def vt_chunk_tiles(ch):
                src_ap = vth[ch] if ch < 2 else vtr[ch - 2]
                tiles = []
                for j in range(CC // VSPLIT):
                    t = vtpool.tile([128, VSPLIT, RC], F8, tag=f"vt{j}")
                    nc.sync.dma_start(
                        out=t, in_=src_ap[:, j * VSPLIT:(j + 1) * VSPLIT, :])
                    tiles.append(t)
                return tiles

            vt_cur = vt_chunk_tiles(0)

            const4 = cpool.tile([1, HID + HID], BF16)
            nc.sync.dma_start(out=const4, in_=packc)

            packf_s = cpool.tile([128, BL + BK], F32R)
            nc.sync.dma_start(out=packf_s, in_=packf)

            vt_next = vt_chunk_tiles(1)

            # ---- on-device constant generation (GpSimd is idle at startup)
            packb_s = cpool.tile([128, HID + HID], BF16)
            w2b_s = packb_s[:, 0:HID]
            b1b_s = packb_s[0:BK, HID:HID + HID]
            nc.gpsimd.partition_broadcast(b1b_s, const4[0:1, HID:HID + HID])
            nc.gpsimd.partition_broadcast(w2b_s, const4[0:1, 0:HID])

            # one-hot row->bk selector: oneh[p, g*36+n] = 1 iff g == p
            oneh_f = cpool.tile([BK, R], F32, name="oneh_f")
            nc.gpsimd.memset(oneh_f, 1.0)
            nc.gpsimd.affine_select(
                oneh_f.rearrange("p (g n) -> p g n", n=N),
                oneh_f.rearrange("p (g n) -> p g n", n=N),
                [[1, BK], [0, N]], mybir.AluOpType.is_equal, 0.0,
                base=0, channel_multiplier=-1)
            oneh_s = cpool.tile([BK, R], BF16)
            nc.scalar.copy(oneh_s, oneh_f)

            selb_s = packf_s[0:BK, 0:BL]
            selbt_s = packf_s[0:BL, BL:BL + BK]

            # per-row logits, laid out [p, rt] with row = rt*128 + p, split
            # into two tiles so the first half's softmax hides under the
            # main loop. 32 columns (StreamTranspose needs 32x32 blocks).
            NRT_A = 18   # 18*128 rows = 64 (b,k) groups — a 32-aligned bk split
            ls_a = cpool.tile([128, 32], F32)
            nc.vector.memset(ls_a, 0.0)
            ls_b = cpool.tile([128, 32], F32)
            nc.vector.memset(ls_b, 0.0)

            lg = dpool.tile([R], F32)
            lg2 = lg.rearrange("(t p) -> t p", t=NRT, p=128)

            hq_s = cpool.tile([BK, HID], BF16)
            s96 = small.tile([BK, N], F32)
            e96 = small.tile([BK, N], F32R)
            sums_ps = pspool.tile([BL, N], F32, tag="sm", bufs=2)

            def flush_logits(ls, ls_t_name, t0, t1, s96_slice):
                # ls[p, t - t0] holds L[t*128 + p] for t in [t0, t1).
                # Build the true transpose ls_t2[t - t0, p] from 4 32x32
                # block transposes, then one SBUF->SBUF DMA re-groups
                # (both streams walk r = t*128+p = bk*36+n sequentially).
                ls_t2 = cpool.tile([32, 128], F32, name=ls_t_name)
                for i in range(4):
                    nc.vector.transpose(
                        ls_t2[0:32, 32 * i:32 * i + 32],
                        ls[32 * i:32 * i + 32, 0:32])
                nc.sync.dma_start(out=lg2[t0:t1, :], in_=ls_t2[0:t1 - t0, :])
                nc.sync.dma_start(
                    out=s96_slice,
                    in_=lg.rearrange("(bk n) -> bk n", n=N)[
                        t0 * 128 // N:t1 * 128 // N, :])

            PV = VSPLIT // 2   # DoubleRow cc-pairs per v DMA tile
            PW = WG // 2       # DoubleRow cc-pairs per wv group

            def emit_vmms(t, ps):
                for cp in range(CC // 2):
                    lhsT = vt_cur[cp // PV][:, (cp % PV) * 2:(cp % PV) * 2 + 2,
                                            t * RT:(t + 1) * RT]
                    wvc = wv_g[cp // PW][:, (cp % PW) * 2:(cp % PW) * 2 + 2, :]
                    mm(ps[:, 0:512], lhsT, wvc[:, :, 0:512],
                       start=(cp == 0), stop=False, perf_mode=DR)
                    mm(ps[:, 512:1024], lhsT, wvc[:, :, 512:1024],
                       start=(cp == 0), stop=False, perf_mode=DR)

            def emit_closer(rt, ps):
                oh = oneh_s[:, rt * RT:(rt + 1) * RT]
                mm(ps[:, 0:512], oh, hq_s[:, 0:512], start=False, stop=True)
                mm(ps[:, 512:1024], oh, hq_s[:, 512:1024], start=False, stop=True)
                relu_w2 = work.tile([128, HID], F32, tag="relu_w2")
                ls, col = (ls_a, rt) if rt < NRT_A else (ls_b, rt - NRT_A)
                nc.vector.scalar_tensor_tensor(
                    out=relu_w2, in0=ps, scalar=0.0, in1=w2b_s,
                    op0=MAX, op1=MULT,
                    accum_out=ls[:, col:col + 1])
                if rt == NRT_A - 1:
                    # flush + start the softmax head for bk rows 0:64 while
                    # the main loop still runs
                    flush_logits(ls_a, "ls_ta", 0, NRT_A, s96[0:64, :])
                    nc.scalar.activation(e96[0:64, :], s96[0:64, :],
                                         mybir.ActivationFunctionType.Exp)
                    mm(sums_ps, selb_s[0:64, :], e96[0:64, :],
                       start=True, stop=False)

            # ---- hq[bk, h] = q @ W1q^T (+ b1*S on eviction), via fp8
            # DoubleRow; first in PE program order so the PE starts as soon
            # as qtwq (the first upload) lands.
            hq_ps = [pspool.tile([BK, 512], F32, tag="sm", bufs=2,
                                 name=f"hq_ps{i}") for i in range(2)]
            for half in range(2):
                hs = slice(half * 512, (half + 1) * 512)
                for cp in range(QC // 2):
                    mm(hq_ps[half],
                       qtwq_s[:, 2 * cp:2 * cp + 2, 0:BK],
                       qtwq_s[:, 2 * cp:2 * cp + 2,
                              BK + half * 512:BK + (half + 1) * 512],
                       start=(cp == 0), stop=(cp == QC // 2 - 1),
                       perf_mode=DR)
                # psum -> SBUF with the b1*S row added
                nc.vector.scalar_tensor_tensor(
                    out=hq_s[:, hs], in0=hq_ps[half], scalar=0.0,
                    in1=b1b_s[:, hs],
                    op0=BYPASS, op1=ADD)

            # ---- chunks 0..8
            for ch in range(NCH):
                if ch + 1 < NCH:
                    if ch > 0:
                        vt_next = vt_chunk_tiles(ch + 1)
                for t in range(NT):
                    rt = ch * NT + t
                    ps = pspool.tile([128, HID], F32, tag="ps", bufs=3)
                    emit_vmms(t, ps)
                    emit_closer(rt, ps)
                vt_cur = vt_next

            # ---- flush remaining logits, finish the softmax
            flush_logits(ls_b, "ls_tb", NRT_A, NRT, s96[64:BK, :])
            nc.scalar.activation(e96[64:BK, :], s96[64:BK, :],
                                 mybir.ActivationFunctionType.Exp)
            mm(sums_ps, selb_s[64:BK, :], e96[64:BK, :],
               start=False, stop=True)
            rcp = small.tile([BL, N], F32R)
            with nc.allow_low_precision(reason="f32r is full fp32 bits"):
                nc.vector.reciprocal(rcp, sums_ps)
            rexp_ps = pspool.tile([BK, N], F32, tag="sm", bufs=2)
            mm(rexp_ps, selbt_s, rcp, start=True, stop=True)
            w96 = small.tile([BK, N], F32)
            nc.vector.scalar_tensor_tensor(
                out=w96, in0=e96, scalar=0.0, in1=rexp_ps,
                op0=BYPASS, op1=MULT)
            nc.sync.dma_start(
                out=out.rearrange("b k n o -> (b k) (n o)"), in_=w96)

    nc.compile()
    return nc


def _get_nc():
    global _NC_CACHE
    if _NC_CACHE is None:
        _NC_CACHE = _build_nc()
    return _NC_CACHE


def _prepare_in_maps(inputs):
    v = np.asarray(inputs["v"], dtype=np.float32)
    q = np.asarray(inputs["q"], dtype=np.float32)
    W1 = np.asarray(inputs["W1"], dtype=np.float32)
    g1 = np.float64(np.asarray(inputs["g1"]))
    b1 = np.asarray(inputs["b1"], dtype=np.float32)
    W2 = np.asarray(inputs["W2"], dtype=np.float32)
    g2 = np.float64(np.asarray(inputs["g2"]))
    # b2 is a scalar added to every logit -> cancels in softmax over k.

    W1e = ((g1 / np.linalg.norm(W1.astype(np.float64))) * W1).astype(np.float32)
    W2e = ((g2 / np.linalg.norm(W2.astype(np.float64))) * W2).astype(np.float32)

    BF = ml_dtypes.bfloat16
    FP8 = ml_dtypes.float8_e4m3
    W1s = (W1e * SCALE).astype(np.float32)
    # partition-major repacks: [..., 128 p, chunk, inner]
    w1vt = np.ascontiguousarray(                       # [128, 16, 1024]
        W1s[:, :VD].T.reshape(CC, 128, HID).transpose(1, 0, 2)).astype(FP8)
    w1qt = W1s[:, VD:].T.reshape(QC, 128, HID).transpose(1, 0, 2)  # [128, 8, 1024]
    selb = (np.arange(BL)[None, :] == (np.arange(BK) // K)[:, None]).astype(np.float32)

    packc = np.zeros((1, HID + HID), dtype=BF)
    packc[0, 0:HID] = (W2e.reshape(HID) / SCALE).astype(BF)
    packc[0, HID:HID + HID] = (b1 * SCALE).astype(BF)
    packf = np.zeros((128, BL + BK), dtype=np.float32)
    packf[0:BK, 0:BL] = selb
    packf[0:BL, BL:BL + BK] = selb.T

    shared = dict(w1vt=w1vt, packc=packc, packf=packf)
    in_maps = []
    for c in range(NCORES):
        vl = v[c * BL:(c + 1) * BL].reshape(R, VD)
        # vt[ch, p, cc, r_in_chunk] = v[ch*RC + r, cc*128 + p]
        vt4 = np.ascontiguousarray(
            vl.T.reshape(CC, 128, NCH, RC).transpose(2, 1, 0, 3)).astype(FP8)
        ql = q[c * BL:(c + 1) * BL].reshape(BK, QD)
        qt3 = ql.T.reshape(QC, 128, BK).transpose(1, 0, 2)   # [128, 8, 96]
        qtwq = np.concatenate([qt3, w1qt], axis=2).astype(FP8)  # [128, 8, 1120]
        in_maps.append(dict(vth=np.ascontiguousarray(vt4[:2]),
                            vtr=np.ascontiguousarray(vt4[2:]),
                            qtwq=np.ascontiguousarray(qtwq), **shared))
    return in_maps


def kernel(**inputs) -> np.ndarray:
    in_maps = _prepare_in_maps(inputs)
    nc = _get_nc()
    res = run_bass_kernel_spmd(nc, in_maps, list(range(NCORES)))
    outs = [res.results[c]["out"].reshape(BL, K, N, 1) for c in range(NCORES)]
    return np.concatenate(outs, axis=0)
